# revision 50
# baseline (speedup 1.0000x reference)
"""Trainium2 Bass kernel for the batched differentiable EKF.

Problem: B=8192 independent rows, T=2048 sequential EKF steps per row
(2-dim state Kalman filter, scalar observation). Output [B, T, 2] f32.

Strategy (v2):
- Data parallel: B sharded 1024 rows/core across 8 cores.
- Time parallel within a core: T split into C chunks of L steps, each
  preceded by a W-step warmup from a cold init (the filter forgets its
  init in ~16-24 steps; W=32 is far below the fp16 noise floor).
  Chunk 0's warmup IS the true filter from the exact reference init.
- Lanes: 128 partitions x GC = G*C lanes (row-group x chunk).
- fp16 everywhere on-chip (measured 3.6e-3 rel err vs 2e-2 gate):
  DVE tensor_tensor on packed fp16 runs in 2x mode (0.52 ns/col).
- Covariance/gain chain on DVE, retimed to a ~6-hop recurrence cycle
  (pp00 = (2*a*p01 + (p00+q)) + a*(a*p11)) to keep the in-order engine
  fed; gains via the production 2-NR reciprocal_approx_fast custom op
  (1-NR was measured 6x worse in the error tail).
- State chain on GPSIMD as plain tensor_tensor ops (the STT/scan
  opcodes are not legal on Pool); the yk pair and the drain-region
  x-steps run on DVE.
- Input derivation (sigmoid/relu/affines) and fp16->f32 output
  conversion on the Scalar engine: q = 0.1*r and r = max(100*sig, 1)
  are affine in relu(100*sig-1), so three Copy/Relu passes derive
  everything.
- Input DMA in 24-step slabs (96B runs, ~180 GB/s); output converted
  and written in 12-step chunks (96B runs).
"""

import numpy as np

import concourse.bass as bass
import concourse.bacc as bacc
import concourse.mybir as mybir
import concourse.tile as tile

F32 = mybir.dt.float32
F16 = mybir.dt.float16
ALU = mybir.AluOpType
ACTF = mybir.ActivationFunctionType
PART = 128

# ----------------------------------------------------------------------
# Custom DVE ops (registered at import; sha computed dynamically)
# ----------------------------------------------------------------------
from concourse.dve_spec import Spec, Src0, Src1, One, Bin, AluOp, lower
import concourse.dve_ops as dve_ops_mod
from concourse.dve_ops import DveOp, OPS
from concourse.dve_uop import DveOpSpec


def _register_dve_op(name: str, spec: Spec) -> DveOp:
    for op in OPS:
        if op.name == name:
            return op
    shas = {}
    for ver in ("v3", "v4"):
        uops = lower(spec, ver=ver)
        shas[ver] = DveOpSpec(name=name, opcode=0, uops=uops, rd1_en=True).sha(ver)
    op = DveOp(name, spec, subdim=False, uops_sha=shas)
    OPS.append(op)
    dve_ops_mod.CUSTOM_DVE_SPECS[name] = spec
    dve_ops_mod._SUB_OPCODE_FOR_NAME[name] = (
        dve_ops_mod._CUSTOM_DVE_ROW_BASE + len(OPS) - 1
    )
    assert dve_ops_mod._SUB_OPCODE_FOR_NAME[name] < 0x20
    return op


# out = in0 * (1 - in1)   (posterior covariance: P' = pp * (1 - K0))
OMK = _register_dve_op(
    "EKF_OMK",
    Spec(
        body=Src0 * (One - Src1),
        reference=lambda in0, in1, s0, s1, imm2: (
            in0 * (1.0 - np.asarray(in1).reshape(np.asarray(in0).shape))
        ).astype(np.float32),
    ),
)

# out ~= 1 / (in0 + in1 + 1): bit-trick seed + Chebyshev + 1 NR step.
# (~0.4% rel err; feeds the Kalman gains, same league as fp16 rounding.)
_R_C0 = -0.23549792
_R_C1 = 2.0017324
_t = (Src0 + Src1) + One


def _recip1_ref(in0, in1, s0, s1, imm2):
    x = (
        np.asarray(in0, np.float32)
        + np.asarray(in1, np.float32).reshape(np.asarray(in0).shape)
        + np.float32(1.0)
    ).astype(np.float32)
    nx = (~x.view(np.uint32)).view(np.float32)
    y0 = nx * np.float32(s0)
    return (y0 * (np.float32(s1) - x * y0)).astype(np.float32)


from concourse.dve_spec import C0 as _C0, C1 as _C1

_y0n = Bin(AluOp.BITWISE_NOT, _t, _t) * _C0
RECIP1P1 = _register_dve_op(
    "EKF_RECIP1P1",
    Spec(body=_y0n * (_C1 - _t * _y0n), reference=_recip1_ref),
)

# out = in0^2 * in1   (t3 = pp01^2 * rr = K1*pp01)
SQM = _register_dve_op(
    "EKF_SQM",
    Spec(
        body=(Src0 * Src0) * Src1,
        reference=lambda in0, in1, s0, s1, imm2: (
            np.asarray(in0, np.float32) ** 2
            * np.asarray(in1, np.float32).reshape(np.asarray(in0).shape)
        ).astype(np.float32),
    ),
)


# ----------------------------------------------------------------------
# Kernel builder (single core, b_loc rows)
# ----------------------------------------------------------------------
def build_core_kernel(
    b_loc: int,
    t_len: int,
    c_chunks: int,
    warm: int,
    slab: int,
    ns_o: int = 16,
    xdelay: int = 8,
    prefetch: int = 22,
):
    G = b_loc // PART
    C = c_chunks
    W = warm
    L = (t_len - W) // C
    assert C * L + W == t_len, (t_len, C, L, W)
    GC = G * C
    steps = W + L
    assert steps % slab == 0
    assert slab % ns_o == 0

    nc = bacc.Bacc("TRN2", target_bir_lowering=False, debug=False)
    pr_h = nc.dram_tensor("price", [b_loc, t_len], F32, kind="ExternalInput")
    hu_h = nc.dram_tensor("hurst", [b_loc, t_len], F32, kind="ExternalInput")
    vs_h = nc.dram_tensor("vol_sigma", [b_loc, t_len], F32, kind="ExternalInput")
    out_h = nc.dram_tensor("out", [b_loc, t_len, 2], F32, kind="ExternalOutput")

    V = None
    GP = None

    def in_slab_src(handle, g, s0, ns):
        # [p, c, s] <- dram[(g*128+p), c*L + s0 + s]
        return bass.AP(
            tensor=handle,
            offset=g * PART * t_len + s0,
            ap=[[t_len, PART], [L, C], [1, ns]],
        )

    def dup_pair(ap2d):
        # [128, N] -> [128, 2, N] zero-stride broadcast (planar dup)
        return ap2d.unsqueeze(1).broadcast_to([PART, 2, ap2d.shape[1]])

    # first regular slab split in two: halves the startup-critical
    # h+sigma DMA span (descriptor count per DMA is ns-independent)
    slabs = [(0, slab // 2), (slab // 2, slab // 2)] + [
        (i * slab, slab) for i in range(1, steps // slab)
    ]
    slab_starts = [s0 for s0, _ in slabs]
    si_of = {}
    for _si, (_s0, _ns) in enumerate(slabs):
        for _s in range(_s0, _s0 + _ns):
            si_of[_s] = _si

    with tile.TileContext(nc) as tc:
        with (
            tc.tile_pool(name="hs", bufs=4) as hsp,      # h/sigma f32 staging
            tc.tile_pool(name="zst", bufs=2) as zsp,     # z fp16 step-major
            tc.tile_pool(name="der", bufs=2) as derp,    # a,q,u fp16 step-major
            tc.tile_pool(name="ovp", bufs=2) as ovp,     # ov16 fp16 x-slab
            tc.tile_pool(name="ofp", bufs=2) as ofp,     # ovf32 DMA staging
            tc.tile_pool(name="st", bufs=3) as stp,      # per-step cov tiles
            tc.tile_pool(name="kkp", bufs=xdelay + 2) as kkp,
            tc.tile_pool(name="ini", bufs=1) as inip,
        ):
            bias_m5 = inip.tile([PART, 1], F32, tag="bm5")
            bias_m1 = inip.tile([PART, 1], F32, tag="bm1")
            bias_h = inip.tile([PART, 1], F32, tag="bh")
            bias_t = inip.tile([PART, 1], F32, tag="bt")
            nc.gpsimd.memset(bias_m5[:], -5.0)
            nc.gpsimd.memset(bias_m1[:], -1.0)
            nc.gpsimd.memset(bias_h[:], 0.5)
            nc.gpsimd.memset(bias_t[:], 0.1)

            # initial state tiles
            p_init = inip.tile([PART, 3 * GC], F16, tag="p0")   # [p00|p01|p11]
            x_init = inip.tile([PART, 2 * GC], F16, tag="x0")   # [x0|x1]
            nc.gpsimd.memset(p_init[:, 0:GC], 1.0)
            nc.gpsimd.memset(p_init[:, GC : 2 * GC], 0.0)
            nc.gpsimd.memset(p_init[:, 2 * GC : 3 * GC], 1.0)

            V = nc.vector
            GP = nc.gpsimd
            SC = nc.scalar

            slab_ctx = {}
            pprev = {}     # views: p00, p01, p11 (planes of prev P tile)
            xprev = {}     # views: x0, x1 ([128, GC] fp16)
            kk_of = {}     # step -> (kk tile, slab index)

            def load_slab(si, stage):
                s0, ns = slabs[si]
                if stage == 0:
                    o_sl = ovp.tile([PART, 2 * ns * GC], F16, tag="o")
                    slab_ctx[si] = dict(
                        s0=s0,
                        ns=ns,
                        o_sl=o_sl,
                        o3=o_sl[:].rearrange(
                            "p (two s gc) -> p two s gc", two=2, s=ns, gc=GC
                        ),
                    )
                    return slab_ctx[si]
                if stage == 3:
                    ctx = slab_ctx[si]
                    z_sl = hsp.tile([PART, GC * ns], F32, tag="hs")
                    z16 = zsp.tile([PART, ns * GC], F16, tag="z16")
                    v4 = z_sl[:].rearrange(
                        "p (g c s) -> p g c s", g=G, c=C, s=ns
                    )
                    for g in range(G):
                        nc.sync.dma_start(v4[:, g], in_slab_src(pr_h, g, s0, ns))
                    # f32 [lane][s] -> f16 step-major [s][lane]
                    SC.copy(
                        out=z16[:].rearrange("p (s gc) -> p gc s", gc=GC, s=ns),
                        in_=z_sl[:].rearrange("p (gc s) -> p gc s", gc=GC, s=ns),
                    )
                    ctx["zs2"] = z16[:].rearrange(
                        "p (s gc) -> p s gc", gc=GC, s=ns
                    )
                    return ctx
                ctx = slab_ctx[si]
                if stage == 1:
                    h_sl = hsp.tile([PART, GC * ns], F32, tag="hs")
                    a_sl = derp.tile([PART, ns * GC], F16, tag="a")
                    v4 = h_sl[:].rearrange(
                        "p (g c s) -> p g c s", g=G, c=C, s=ns
                    )
                    for g in range(G):
                        nc.sync.dma_start(v4[:, g], in_slab_src(hu_h, g, s0, ns))
                    h_v = h_sl[:].rearrange("p (gc s) -> p gc s", gc=GC, s=ns)
                    a_v = a_sl[:].rearrange("p (s gc) -> p gc s", gc=GC, s=ns)
                    # A = sigmoid(10h-5), then in-place a = 0.5*A + 0.5
                    SC.activation(
                        a_v, h_v, ACTF.Sigmoid, bias=bias_m5[:], scale=10.0
                    )
                    SC.activation(a_sl[:], a_sl[:], ACTF.Copy, bias=0.5, scale=0.5)
                    ctx["a2"] = a_sl[:].rearrange(
                        "p (s gc) -> p s gc", gc=GC, s=ns
                    )
                    return ctx
                v_sl = hsp.tile([PART, GC * ns], F32, tag="hs")
                q_sl = derp.tile([PART, ns * GC], F16, tag="q")
                r_sl = derp.tile([PART, ns * GC], F16, tag="u")
                v4 = v_sl[:].rearrange("p (g c s) -> p g c s", g=G, c=C, s=ns)
                for g in range(G):
                    nc.sync.dma_start(v4[:, g], in_slab_src(vs_h, g, s0, ns))
                v_v = v_sl[:].rearrange("p (gc s) -> p gc s", gc=GC, s=ns)
                r_v = r_sl[:].rearrange("p (s gc) -> p gc s", gc=GC, s=ns)
                # r1 = relu(100*sig - 1) + 1 = max(100*sig, 1); q = 0.1*r1
                if si == 0:
                    # startup-latency path: one DVE op each instead of the
                    # 3-deep serial ACT chain
                    V.tensor_scalar(r_v, v_v, 100.0, 1.0, ALU.mult, ALU.max)
                    V.tensor_scalar(q_sl[:], r_sl[:], 0.1, 0.0, ALU.mult, ALU.add)
                else:
                    SC.activation(r_v, v_v, ACTF.Relu, bias=bias_m1[:], scale=100.0)
                    SC.activation(r_sl[:], r_sl[:], ACTF.Copy, bias=1.0, scale=1.0)
                    SC.activation(q_sl[:], r_sl[:], ACTF.Copy, bias=0.0, scale=0.1)
                ctx["q2"] = q_sl[:].rearrange("p (s gc) -> p s gc", gc=GC, s=ns)
                ctx["r2"] = r_sl[:].rearrange("p (s gc) -> p s gc", gc=GC, s=ns)
                return ctx

            def emit_p(gs, si):
                sl = slab_ctx[si]
                s = gs - sl["s0"]
                A = sl["a2"][:, s]            # [128, GC] fp16 packed
                Q = sl["q2"][:, s]
                R1 = sl["r2"][:, s]
                p00 = pprev["p00"]
                p01 = pprev["p01"]
                p11 = pprev["p11"]

                t1 = stp.tile([PART, GC], F16, tag="t1")
                m1 = stp.tile([PART, GC], F16, tag="m1")
                m2 = stp.tile([PART, GC], F16, tag="m2")
                s1 = stp.tile([PART, GC], F16, tag="s1")
                sq = stp.tile([PART, GC], F16, tag="sq")
                pq = stp.tile([PART, 2 * GC], F16, tag="pq")   # [pq|pq11]
                pp = stp.tile([PART, 2 * GC], F16, tag="pp")   # [pp00|pp01]
                rr = stp.tile([PART, GC], F16, tag="rr")
                sS = stp.tile([PART, GC], F16, tag="sS")
                t3 = stp.tile([PART, GC], F16, tag="t3")
                pn = stp.tile([PART, 3 * GC], F16, tag="pn")
                kk = kkp.tile([PART, 2 * GC], F16, tag="kk")   # [K0|K1]

                pp00v = pp[:][:, 0:GC]
                pp01v = pp[:][:, GC : 2 * GC]
                ppv = pp[:].rearrange("p (two gc) -> p two gc", two=2)
                pqv = pq[:].rearrange("p (two gc) -> p two gc", two=2)
                kkv = kk[:].rearrange("p (two gc) -> p two gc", two=2)

                # retimed 6-hop recurrence cycle, emitted so dependent
                # pairs are separated by independent ops (in-order engine):
                # pp00 = (2*a*p01 + (p00+q)) + a*(a*p11); t3 = pp01^2 * rr
                p02 = pprev["p02"]  # [128, 2, GC] planes {0,2} view
                V.tensor_tensor(out=t1[:], in0=A, in1=p11, op=ALU.mult)
                V.tensor_tensor(out=m1[:], in0=A, in1=p01, op=ALU.mult)
                V.tensor_tensor(out=pqv, in0=p02, in1=dup_pair(Q), op=ALU.add)
                V.tensor_tensor(out=m2[:], in0=A, in1=t1[:], op=ALU.mult)
                V.tensor_tensor(out=pp01v, in0=p01, in1=t1[:], op=ALU.add)
                V.scalar_tensor_tensor(
                    out=s1[:], in0=m1[:], scalar=2.0, in1=pqv[:, 0],
                    op0=ALU.mult, op1=ALU.add,
                )
                V.tensor_tensor(
                    out=pp00v, in0=s1[:], in1=m2[:], op=ALU.add
                )
                GP.tensor_tensor(out=sq[:], in0=pp01v, in1=pp01v, op=ALU.mult)
                V.tensor_tensor(out=sS[:], in0=pp00v, in1=R1, op=ALU.add)
                from concourse.dve_ops import (
                    RECIP_APPROX_FAST_CONSTS as _RC,
                    RECIPROCAL_APPROX_FAST as _RF,
                )
                V._custom_dve(
                    _RF, out=rr[:], in0=sS[:],
                    s0=_RC["s0"], s1=_RC["s1"], imm2=_RC["imm2"],
                )
                V.tensor_tensor(out=kkv, in0=ppv, in1=dup_pair(rr[:]), op=ALU.mult)
                V.tensor_tensor(out=t3[:], in0=sq[:], in1=rr[:], op=ALU.mult)
                pnv = pn[:].rearrange("p (three gc) -> p three gc", three=3)
                V._custom_dve(
                    OMK,
                    out=pnv[:, 0:2],
                    in0=ppv,
                    in1=dup_pair(kkv[:, 0]),
                )
                V.tensor_tensor(
                    out=pnv[:, 2], in0=pqv[:, 1], in1=t3[:], op=ALU.subtract
                )
                pprev["p00"] = pn[:][:, 0:GC]
                pprev["p01"] = pn[:][:, GC : 2 * GC]
                pprev["p11"] = pn[:][:, 2 * GC : 3 * GC]
                p3 = pn[:].rearrange("p (three gc) -> p three gc", three=3)
                # planes {0,2} strided pair view for next PQ
                pn_flat = pn[:]
                pprev["p02"] = bass.AP(
                    tensor=pn_flat.tensor,
                    offset=pn_flat.offset,
                    ap=[list(x) for x in pn_flat.ap[:1]]
                    + [[2 * GC, 2], [1, GC]],
                )
                kk_of[gs] = (kk, si)

            def emit_x(gs, si):
                sl = slab_ctx[si]
                s = gs - sl["s0"]
                A = sl["a2"][:, s]
                Z = sl["zs2"][:, s]           # [128, GC] fp16 packed
                kk, _ = kk_of.pop(gs)
                kkv = kk[:].rearrange("p (two gc) -> p two gc", two=2)
                x0p = xprev["x0"]
                x1p = xprev["x1"]

                t4 = stp.tile([PART, GC], F16, tag="t4")
                xp = stp.tile([PART, GC], F16, tag="xp")
                yy = stp.tile([PART, GC], F16, tag="yy")
                yk = stp.tile([PART, 2 * GC], F16, tag="yk")
                ykv = yk[:].rearrange("p (two gc) -> p two gc", two=2)
                ov = sl["o3"]

                if False:
                    # drain region: P-chain is done, DVE is idle -> run the
                    # x-chain there (2.3x faster per step than Pool)
                    V.tensor_tensor(out=t4[:], in0=A, in1=x1p, op=ALU.mult)
                    V.tensor_tensor(out=xp[:], in0=x0p, in1=t4[:], op=ALU.add)
                    V.tensor_tensor(out=yy[:], in0=Z, in1=xp[:], op=ALU.subtract)
                    V.tensor_tensor(
                        out=ykv, in0=kkv, in1=dup_pair(yy[:]), op=ALU.mult
                    )
                    V.tensor_tensor(
                        out=ov[:, 0, s], in0=xp[:], in1=ykv[:, 0], op=ALU.add
                    )
                    V.tensor_tensor(
                        out=ov[:, 1, s], in0=x1p, in1=ykv[:, 1], op=ALU.add
                    )
                else:
                    GP.tensor_tensor(out=t4[:], in0=A, in1=x1p, op=ALU.mult)
                    GP.tensor_tensor(out=xp[:], in0=x0p, in1=t4[:], op=ALU.add)
                    GP.tensor_tensor(out=yy[:], in0=Z, in1=xp[:], op=ALU.subtract)
                    V.tensor_tensor(
                        out=ykv, in0=kkv, in1=dup_pair(yy[:]), op=ALU.mult
                    )
                    GP.tensor_tensor(
                        out=ov[:, 0, s], in0=xp[:], in1=ykv[:, 0], op=ALU.add
                    )
                    GP.tensor_tensor(
                        out=ov[:, 1, s], in0=x1p, in1=ykv[:, 1], op=ALU.add
                    )
                xprev["x0"] = ov[:, 0, s]
                xprev["x1"] = ov[:, 1, s]

                # flush finished output chunks (fp16 -> f32 + DMA)
                s0 = sl["s0"]
                gstep = s0 + s
                half = ns_o // 2
                if gstep >= steps - ns_o:
                    # drain region: halve the flush so conversion + DMA of
                    # the first half overlaps the last x-steps
                    if (gstep + 1) % half == 0:
                        flush_chunk(si, s - half + 1, half)
                elif (gstep + 1) % ns_o == 0:
                    c0 = s - ns_o + 1         # local chunk start
                    flush_chunk(si, c0, ns_o)

            def flush_chunk(si, c0, n):
                """Convert ov16[c0:c0+n] to f32 and DMA to DRAM."""
                sl = slab_ctx[si]
                s0 = sl["s0"]
                g0 = s0 + c0              # global step of chunk start
                if g0 < W < g0 + n:
                    # split at the warmup boundary
                    flush_chunk(si, c0, W - g0)
                    flush_chunk(si, c0 + (W - g0), n - (W - g0))
                    return
                if g0 + n <= W:
                    # pure warmup: only chunk 0 of each g produces output
                    of = ofp.tile([PART, G * n * 2], F32, tag="ow")
                    ofv = of[:].rearrange(
                        "p (g s two) -> p g s two", g=G, s=n, two=2
                    )
                    src = sl["o_sl"][:].rearrange(
                        "p (two s g c) -> p g s two c", two=2, s=sl["ns"],
                        g=G, c=C,
                    )[:, :, c0 : c0 + n, :, 0]
                    SC.copy(out=ofv, in_=src)
                    dst = bass.AP(
                        tensor=out_h,
                        offset=g0 * 2,
                        ap=[[t_len * 2, PART], [PART * t_len * 2, G],
                            [1, n * 2]],
                    )
                    nc.sync.dma_start(
                        dst, of[:].rearrange("p (g sx) -> p g sx", g=G)
                    )
                    return
                # full output chunk: all lanes
                of = ofp.tile([PART, GC * n * 2], F32, tag="of")
                ofv = of[:].rearrange(
                    "p (g c s two) -> p g c s two", g=G, c=C, s=n, two=2
                )
                src = sl["o_sl"][:].rearrange(
                    "p (two s g c) -> p g c s two", two=2, s=sl["ns"], g=G, c=C
                )[:, :, :, c0 : c0 + n, :]
                SC.copy(out=ofv, in_=src)
                of3 = of[:].rearrange(
                    "p (g c sx) -> p g c sx", g=G, c=C, sx=n * 2
                )
                for g in range(G):
                    dst = bass.AP(
                        tensor=out_h,
                        offset=g * PART * t_len * 2 + g0 * 2,
                        ap=[[t_len * 2, PART], [L * 2, C], [1, n * 2]],
                    )
                    nc.sync.dma_start(dst, of3[:, g])

            # ---------------- main loop ----------------
            for gs in range(steps + xdelay):
                if gs < steps:
                    if gs == 0:
                        ctx = load_slab(0, 0)
                        load_slab(0, 1)
                        load_slab(0, 2)
                        load_slab(0, 3)
                        for st4 in (0, 1, 2, 3):
                            load_slab(1, st4)
                        # x init from z: x0 = z[:,0], x1 = z[:,1]-z[:,0]
                        zs2 = ctx["zs2"]
                        xiv = x_init[:].rearrange(
                            "p (two gc) -> p two gc", two=2
                        )
                        # on Pool: keeps z-dependent ops out of the DVE
                        # in-order queue (z lands ~30us after launch)
                        GP.tensor_scalar_mul(xiv[:, 0], zs2[:, 0], 1.0)
                        GP.tensor_tensor(
                            out=xiv[:, 1], in0=zs2[:, 1], in1=zs2[:, 0],
                            op=ALU.subtract,
                        )
                        pprev["p00"] = p_init[:][:, 0:GC]
                        pprev["p01"] = p_init[:][:, GC : 2 * GC]
                        pprev["p11"] = p_init[:][:, 2 * GC : 3 * GC]
                        pi = p_init[:]
                        pprev["p02"] = bass.AP(
                            tensor=pi.tensor,
                            offset=pi.offset,
                            ap=[list(x) for x in pi.ap[:1]]
                            + [[2 * GC, 2], [1, GC]],
                        )
                        xprev["x0"] = xiv[:, 0]
                        xprev["x1"] = xiv[:, 1]
                    for stage, lead in ((0, prefetch), (3, prefetch),
                                        (1, prefetch - 3), (2, prefetch - 6)):
                        nxt = gs + lead
                        if nxt in slab_starts and nxt < steps:
                            _si = slab_starts.index(nxt)
                            if _si >= 2:
                                load_slab(_si, stage)
                    emit_p(gs, si_of[gs])
                xg = gs - xdelay
                if xg >= 0:
                    emit_x(xg, si_of[xg])
    nc.compile()
    return nc


# ----------------------------------------------------------------------
# Full-problem entry point
# ----------------------------------------------------------------------
B, T = 8192, 2048
NCORES = 8
B_LOC = B // NCORES
C_CHUNKS = 18
WARM = 32
SLAB = 24
NS_O = 12

_nc_cache = {}


def _get_nc():
    key = (B_LOC, T, C_CHUNKS, WARM, SLAB, NS_O)
    if key not in _nc_cache:
        _nc_cache[key] = build_core_kernel(*key)
    return _nc_cache[key]


def kernel(price: np.ndarray, hurst: np.ndarray, vol_sigma: np.ndarray) -> np.ndarray:
    from concourse import bass_utils

    price = np.ascontiguousarray(price, dtype=np.float32)
    hurst = np.ascontiguousarray(hurst, dtype=np.float32)
    vol_sigma = np.ascontiguousarray(vol_sigma, dtype=np.float32)
    nc = _get_nc()
    in_maps = []
    for k in range(NCORES):
        sl = slice(k * B_LOC, (k + 1) * B_LOC)
        in_maps.append(
            {
                "price": price[sl],
                "hurst": hurst[sl],
                "vol_sigma": vol_sigma[sl],
            }
        )
    res = bass_utils.run_bass_kernel_spmd(
        nc, in_maps, core_ids=list(range(NCORES))
    )
    return np.concatenate([r["out"] for r in res.results], axis=0)
